# revision 55
# baseline (speedup 1.0000x reference)
"""Trainium2 Bass kernel for GQA attention with RoPE, causal mask, and
attention sinks (nn_Attention_65094524338392).

Sharding: tensor-parallel by heads across 8 NeuronCores. Core c owns query
heads 4c..4c+3 and kv-head c (NREP=4). Each core computes QKV projections
over the full sequence for its heads directly in [hd, seq] layout
(weight-stationary matmuls against host-pretransposed x^T), flash-style
causal attention, then two chunked AllToAlls (one per head pair)
redistribute attention outputs from head-sharding to sequence-sharding so
each core computes the output projection for its 256-row sequence slice.
The first AllToAll overlaps with attention for the second head pair.

RoPE in [hd, seq] layout: rot(x) is a fixed 128-row permutation (swap
32-row halves within each head), computed as a permutation matmul on the
PE; then q = x*cosT + rot*sinsT with per-(row,col) cos/sin tables (sign
of sin folded into sinsT).

Math note: the sink scaling folds into the softmax normalizer:
    out = (sum_k exp(s_k) v_k) / (sum_k exp(s_k) + exp(sink))
so no logs/sigmoids are needed on device; |s| is small enough that no
max-subtraction is needed for exp stability.

Phases run sequentially and dense (no interleave): gaps in the PE stream
re-throttle the HAM clock gate (2.4 -> 1.2 GHz), so each phase boundary
re-pins the clock with a short burst of dummy matmuls.
"""

import os
import sys

sys.path.insert(0, "/opt/trn_rl_repo")

import ml_dtypes
import numpy as np

import concourse.bass as bass
import concourse.mybir as mybir
import concourse.tile as tile
from concourse import bacc
from concourse.bass_utils import run_bass_kernel_spmd

# Problem shapes
B, S, DIM = 1, 2048, 2048
NH, NKV, HD = 32, 8, 64
NREP = NH // NKV
SCALE = 1.0 / float(np.sqrt(HD))
NCORES = 8
HPC = NH // NCORES            # query heads per core (4)
SB = 512                      # seq block (attention q-block)
NSB = S // SB                 # 4
NT = S // 128                 # 16 seq tiles
ND = DIM // 128               # 16 contraction tiles
MYS = S // NCORES             # output rows per core (256)

# packed consts layout (bf16, 128 partitions): offsets in the free dim
CO_IDENT = 0
CO_PERM = 128
CO_MASKD = 256
CO_BIAS = 384          # [128, 16]; cols 0..2 = per-partition bias per chunk
CO_COS = 400
CO_SINS = 400 + S
CO_WOB = 400 + 2 * S   # [128, DIM]; wob broadcast across partitions
CO_END = 400 + 2 * S + DIM  # 6496

F32 = mybir.dt.float32
BF16 = mybir.dt.bfloat16

_cache = {}

last_exec_time_ns = None


def _install_ntff_shim():
    """Register the NTFF profile hook so trace=True yields exec_time_ns."""
    import types
    if "antenv.axon_hooks" in sys.modules:
        return
    import antenv
    mod = types.ModuleType("antenv.axon_hooks")
    mod._hook = None
    mod.set_axon_ntff_profile_hook = lambda h: setattr(mod, "_hook", h)
    mod.get_axon_ntff_profile_hook = lambda: mod._hook
    sys.modules["antenv.axon_hooks"] = mod
    antenv.axon_hooks = mod
    from trn_agent_boot.trn_boot import _ntff_profile_via_ctypes
    hook = _ntff_profile_via_ctypes("/opt/axon/libaxon_pjrt.so")
    if hook is not None:
        mod._hook = hook


def _build():
    nc = bacc.Bacc("TRN2", target_bir_lowering=False, debug=False,
                   num_devices=NCORES)

    # Input staging happens in declaration order; the packed consts and
    # wqkv come first, then x^T split by seq block so the first projection
    # can start early; woT (needed last) goes last.
    sinks_e = nc.declare_dram_parameter("sinks4", [1, HPC], F32, isOutput=False)
    biasf_e = nc.declare_dram_parameter("biasf", [128, 4], F32, isOutput=False)
    wqkvT_e = nc.declare_dram_parameter("wqkvT", [128, 3 * ND * 128], BF16,
                                        isOutput=False)
    xT_es = [nc.declare_dram_parameter(f"xT{s}", [128, ND * SB], BF16,
                                       isOutput=False) for s in range(NSB)]
    consts_e = nc.declare_dram_parameter("consts", [128, CO_END], BF16,
                                         isOutput=False)
    woT_e = nc.declare_dram_parameter("woT", [128, ND * DIM], BF16, isOutput=False)
    out_e = nc.declare_dram_parameter("out", [MYS, DIM], BF16, isOutput=True)

    with tile.TileContext(nc) as tc:
        with tc.tile_pool(name="const", bufs=1) as cp, \
             tc.tile_pool(name="xT", bufs=2) as xtp, \
             tc.tile_pool(name="qk", bufs=3) as qkp, \
             tc.tile_pool(name="rope", bufs=2) as rp, \
             tc.tile_pool(name="pt", bufs=3) as ptp, \
             tc.tile_pool(name="ep", bufs=2) as epp, \
             tc.tile_pool(name="fin", bufs=3) as fnp, \
             tc.tile_pool(name="dram", bufs=1, space="DRAM") as dp:

            # ---- constants: one packed load + the tiny fp32 sinks ----
            sinks_sb = cp.tile([1, HPC], F32)
            nc.sync.dma_start(sinks_sb[:], sinks_e[:])
            biasf_sb = cp.tile([128, 4], F32)
            nc.sync.dma_start(biasf_sb[:], biasf_e[:])
            wqkv_sb = cp.tile([128, 3, ND, 128], BF16)
            nc.sync.dma_start(wqkv_sb[:], wqkvT_e[:].rearrange(
                "p (c d f) -> p c d f", c=3, d=ND))
            # x^T per seq block; first 2 issued up front (ahead of the
            # consts pack — rope tables are needed a few us after the
            # first projection starts), the rest inside the projection
            # loop (2 bufs, one block of prefetch)
            xT_s = []
            for s in range(NSB):
                t = xtp.tile([128, ND, SB], BF16, tag="xT")
                if s < 2:
                    nc.sync.dma_start(t[:], xT_es[s][:].rearrange(
                        "p (d f) -> p d f", d=ND))
                xT_s.append(t)
            consts_sb = cp.tile([128, CO_END], BF16)
            nc.sync.dma_start(consts_sb[:], consts_e[:])
            ident_sb = consts_sb[:, CO_IDENT:CO_IDENT + 128]
            perm_sb = consts_sb[:, CO_PERM:CO_PERM + 128]
            maskd_sb = consts_sb[:, CO_MASKD:CO_MASKD + 128]
            cosT_sb = consts_sb[:, CO_COS:CO_COS + S]
            sinsT_sb = consts_sb[:, CO_SINS:CO_SINS + S]
            wobbc_sb = consts_sb[:, CO_WOB:CO_WOB + DIM]

            es_sb = cp.tile([1, HPC], F32)
            nc.scalar.activation(es_sb[:], sinks_sb[:],
                                 mybir.ActivationFunctionType.Exp)
            ones512 = cp.tile([1, 512], BF16)
            nc.gpsimd.memset(ones512[:], 1.0)

            # PE warm-up: a gapless burst of dummy matmuls during the
            # initial input DMAs releases the HAM clock throttle (1.2 ->
            # 2.4 GHz) before real work begins. No data dependencies.
            warm_sb = cp.tile([128, 512], BF16)
            nc.gpsimd.memset(warm_sb[:], 0.0)
            with tc.tile_pool(name="warm", bufs=1, space="PSUM") as wpp:
                warm_ps = wpp.tile([128, 512], F32, tag="warm")
                for _ in range(16):
                    nc.tensor.matmul(warm_ps[:], warm_sb[:, 0:128],
                                     warm_sb[:], start=True, stop=True)

            # persistent activations
            qP = [cp.tile([128, S], BF16, name=f"qP{g}") for g in range(HPC // 2)]
            kTd = cp.tile([128, S], BF16)
            # v padded to 128 columns (ones col + zeros) so the PV matmul's
            # stationary load is FWL-eligible (NumWeights==128)
            v_sb = cp.tile([128, NT, 128], BF16)
            nc.gpsimd.memset(v_sb[:, :, HD:HD + 1], 1.0)
            nc.gpsimd.memset(v_sb[:, :, HD + 1:], 0.0)
            oT = [cp.tile([128, S], BF16, name=f"oT{g}") for g in range(HPC // 2)]
            woT_sb = cp.tile([128, ND, DIM], BF16)

            a2a_in = [dp.tile([S // 2, MYS], BF16, name=f"a2ai{g}")
                      for g in range(2)]
            a2a_out = [dp.tile([S // 2, MYS], BF16, name=f"a2ao{g}")
                      for g in range(2)]

            # ---- phase B: QKV projections + rope -> qP/kTd/v_sb ----
            with tc.tile_pool(name="ppB", bufs=1, space="PSUM") as ppB:
                # dependency-gated warm-up: fires as soon as the first
                # x^T tile lands so QKV starts at the warm PE clock
                warm2 = ppB.tile([128, 512], F32, tag="rot", bufs=2)
                for _ in range(12):
                    nc.tensor.matmul(warm2[:], xT_s[0][:, 0, 0:128],
                                     warm_sb[:], start=True, stop=True)
                for s in range(NSB):
                    if s < 2:
                        nc.sync.dma_start(
                            xT_s[s + 2][:], xT_es[s + 2][:].rearrange(
                                "p (d f) -> p d f", d=ND))
                    sl = slice(s * SB, (s + 1) * SB)
                    for c in range(3):
                        acc = ppB.tile([128, SB], F32, tag="acc", bufs=3)
                        for d in range(ND):
                            nc.tensor.matmul(acc[:], wqkv_sb[:, c, d, :],
                                             xT_s[s][:, d, :],
                                             start=(d == 0),
                                             stop=(d == ND - 1))
                        # bias add (per-partition) fused into the PSUM->SBUF
                        # move on the DVE
                        qk = qkp.tile([128, SB], BF16, tag="qk")
                        nc.vector.tensor_scalar_add(qk[:], acc[:],
                                                    biasf_sb[:, c:c + 1])
                        # rope: rot = perm @ qk (PE), q = qk*cos + rot*sins
                        rot = ppB.tile([128, 512], F32, tag="rot", bufs=2)
                        nc.tensor.matmul(rot[:], perm_sb, qk[:],
                                         start=True, stop=True)
                        t1 = rp.tile([128, SB], BF16, tag="t1")
                        nc.vector.tensor_tensor(t1[:], qk[:], cosT_sb[:, sl],
                                                mybir.AluOpType.mult)
                        t2 = rp.tile([128, SB], BF16, tag="t2")
                        nc.vector.tensor_tensor(t2[:], rot[:], sinsT_sb[:, sl],
                                                mybir.AluOpType.mult)
                        if c < 2:
                            nc.vector.tensor_tensor(qP[c][:, sl], t1[:], t2[:],
                                                    mybir.AluOpType.add)
                        else:
                            nc.vector.tensor_tensor(
                                kTd[0:HD, sl], t1[0:HD, :], t2[0:HD, :],
                                mybir.AluOpType.add)
                            nc.vector.tensor_copy(kTd[HD:128, sl],
                                                  kTd[0:HD, sl])
                            # v: transpose rows 64:128 of qk into [seq, hd]
                            vtg = ppB.tile([128, 256], BF16, tag="vt", bufs=2)
                            for tt in range(4):
                                nc.tensor.transpose(
                                    vtg[:, tt * 64:(tt + 1) * 64],
                                    qk[HD:128, tt * 128:(tt + 1) * 128],
                                    ident_sb[HD:128, HD:128])
                            nc.vector.tensor_copy(
                                v_sb[:, 4 * s:4 * s + 4, 0:HD],
                                vtg[:].rearrange("p (t f) -> p t f", t=4))

            def emit_pv(p0, n_kt, s, pvs, pts):
                """PV matmuls for the pair of p-iterations (p0, p0+1)."""
                for z in range(2):
                    for pp in (p0, p0 + 1):
                        for half in range(2):
                            i = 2 * pp + half
                            st = 128 * (i - 4 * s) if i >= 4 * s else 0
                            nc.tensor.matmul(
                                pvs[z][:, st:512],
                                v_sb[:, i, :],
                                pts[pp][:, z, half * 512 + st:
                                        (half + 1) * 512],
                                start=(i == 0),
                                stop=(i == n_kt - 1))

            def attn_block(ppC, g, s, warm_n):
                """Attention for head pair g over q block s."""
                n_kt = 4 * (s + 1)
                sl = slice(s * SB, (s + 1) * SB)
                if warm_n:
                    # re-pin the warm PE clock across the phase boundary
                    warm_w = ppC.tile([128, 1024], F32, tag="sc", bufs=3)
                    for _ in range(warm_n):
                        nc.tensor.matmul(warm_w[:, 0:512], warm_sb[:, 0:128],
                                         warm_sb[:], start=True, stop=True)
                pvs = [ppC.tile([128, 512], F32, tag="pv", bufs=2,
                                name=f"pv{g}_{s}_{z}") for z in range(2)]
                pts = []
                for p in range(n_kt // 2):
                    scs = []
                    for z in range(2):
                        sc = ppC.tile([128, 1024], F32, tag="sc", bufs=3,
                                      name=f"sc{g}_{s}_{p}_{z}")
                        for half in range(2):
                            i = 2 * p + half
                            st = 128 * (i - 4 * s) if i >= 4 * s else 0
                            nc.tensor.matmul(
                                sc[:, half * 512 + st:(half + 1) * 512],
                                kTd[z * HD:(z + 1) * HD,
                                    i * 128:(i + 1) * 128],
                                qP[g][z * HD:(z + 1) * HD,
                                      s * SB + st:(s + 1) * SB],
                                start=True, stop=True,
                                tile_position=(z * HD, 0))
                        scs.append(sc)
                    pt = ptp.tile([128, 2, 1024], BF16, tag="pt")
                    for z in range(2):
                        if p == 2 * s + 1:
                            # both halves heavily column-restricted: exp
                            # only the computed regions (saves ACT time)
                            for half in range(2):
                                i = 2 * p + half
                                st = 128 * (i - 4 * s)
                                nc.scalar.activation(
                                    pt[:, z, half * 512 + st:(half + 1) * 512],
                                    scs[z][:, half * 512 + st:(half + 1) * 512],
                                    mybir.ActivationFunctionType.Exp,
                                    scale=SCALE)
                        else:
                            nc.scalar.activation(
                                pt[:, z, :], scs[z][:],
                                mybir.ActivationFunctionType.Exp, scale=SCALE)
                    for z in range(2):
                        for half in range(2):
                            i = 2 * p + half
                            if i >= 4 * s:
                                # on GpSimd: frees the DVE for the epilogue
                                st = 128 * (i - 4 * s)
                                nc.gpsimd.tensor_tensor(
                                    pt[:, z, half * 512 + st:
                                       half * 512 + st + 128],
                                    pt[:, z, half * 512 + st:
                                       half * 512 + st + 128],
                                    maskd_sb,
                                    mybir.AluOpType.mult)
                    pts.append(pt)
                    if p % 2 == 1:
                        emit_pv(p - 1, n_kt, s, pvs, pts)

                for z in range(2):
                    h = 2 * g + z
                    pv = pvs[z]
                    # epilogue: out_h = pv[0:64] / (S_row + exp(sink_h))
                    srow = epp.tile([1, 512], F32, tag="srow", bufs=2)
                    nc.vector.scalar_tensor_tensor(
                        srow[:], pv[HD:HD + 1, :], es_sb[0:1, h:h + 1],
                        ones512[:], mybir.AluOpType.add,
                        mybir.AluOpType.mult)
                    rrow = epp.tile([1, 512], F32, tag="rrow", bufs=2)
                    nc.vector.reciprocal_approx_fast(rrow[:], srow[:])
                    rbc = epp.tile([HD, 512], F32, tag="rbc", bufs=3)
                    nc.gpsimd.partition_broadcast(rbc[:], rrow[0:1, :])
                    nc.vector.tensor_tensor(
                        oT[g][z * HD:(z + 1) * HD, sl], pv[0:HD, :],
                        rbc[:], mybir.AluOpType.mult)
                    # stream this (head, block) slice into its A2A shard
                    nc.sync.dma_start(
                        a2a_in[g][:].rearrange(
                            "(j zz p) n -> zz p j n",
                            j=NCORES, zz=2)[z][:, 2 * s:2 * s + 2],
                        oT[g][z * HD:(z + 1) * HD, sl].rearrange(
                            "p (j n) -> p j n", j=2))

            # ---- phase C: attention, head pair 0 then head pair 1 ----
            ag_sb = [cp.tile([128, NCORES, MYS], BF16, name=f"ag{g}")
                     for g in range(2)]
            with tc.tile_pool(name="ppC", bufs=1, space="PSUM") as ppC:
                for s in range(NSB):
                    attn_block(ppC, 0, s, 20 if s == 0 else 0)

                # first AllToAll: heads 0,1 -> sequence shards; overlaps
                # with attention for head pair 1
                nc.gpsimd.collective_compute(
                    "AllToAll", mybir.AluOpType.bypass,
                    replica_groups=[list(range(NCORES))],
                    ins=[a2a_in[0].opt()], outs=[a2a_out[0].opt()])
                nc.sync.dma_start(ag_sb[0][:], a2a_out[0][:].rearrange(
                    "(o p) n -> p o n", p=128))

                # load wo^T now on the SWDGE queue (idle; needed only by
                # the output projection)
                nc.gpsimd.dma_start(woT_sb[:], woT_e[:].rearrange(
                    "p (o f) -> p o f", o=ND))

                for s in range(NSB):
                    attn_block(ppC, 1, s, 6 if s == 0 else 0)

            nc.gpsimd.collective_compute(
                "AllToAll", mybir.AluOpType.bypass,
                replica_groups=[list(range(NCORES))],
                ins=[a2a_in[1].opt()], outs=[a2a_out[1].opt()])
            ago = a2a_out[1][:].rearrange("(o p) n -> p o n", p=128)
            for q in range(4):
                nc.sync.dma_start(ag_sb[1][:, 2 * q:2 * q + 2, :],
                                  ago[:, 2 * q:2 * q + 2, :])

            # ---- output projection for my sequence slice ----
            # contraction rows: kt = 2*j + g -> a2a_out[g] shard j.
            # Two passes over kt parity: the even pass (head pair 0, from
            # the first AllToAll) runs during the second AllToAll's entry
            # barrier; the odd pass runs once its readback lands. All 8
            # accumulation groups hold their PSUM banks across the passes.
            with tc.tile_pool(name="ppD", bufs=1, space="PSUM") as ppD:
                fps = {}
                for m in range(MYS // 128):
                    for n in range(DIM // 512):
                        fps[(m, n)] = ppD.tile([128, 512], F32, tag="fp",
                                               bufs=8, name=f"fp{m}_{n}")
                # re-warm the PE into the first two banks (results are
                # discarded by the start=True of each even pass)
                for mn in ((0, 0), (0, 1)):
                    for _ in range(7):
                        nc.tensor.matmul(fps[mn][:], warm_sb[:, 0:128],
                                         warm_sb[:], start=True, stop=True)
                # even pass: head-pair-0 contraction tiles (first AllToAll)
                for m in range(MYS // 128):
                    for n in range(DIM // 512):
                        fp = fps[(m, n)]
                        for j in range(NCORES):
                            nc.tensor.matmul(
                                fp[:],
                                ag_sb[0][:, j, m * 128:(m + 1) * 128],
                                woT_sb[:, 2 * j, n * 512:(n + 1) * 512],
                                start=(j == 0), stop=False)
                # keep the PE clock warm across the second AllToAll wait:
                # accumulate +0 (zero stationary) into an open group; the
                # second batch gates on the first readback chunk so it runs
                # right before the odd pass
                for _ in range(20):
                    nc.tensor.matmul(fps[(0, 0)][:], warm_sb[:, 0:128],
                                     warm_sb[:], start=False, stop=False,
                                     skip_group_check=True)
                for _ in range(16):
                    nc.tensor.matmul(
                        fps[(0, 1)][:], warm_sb[:, 0:128],
                        ag_sb[1][:, 0:2, :].rearrange("p a b -> p (a b)"),
                        start=False, stop=False, skip_group_check=True)
                # odd pass: (m,n) outer so each group's bias-add + store
                # overlaps the remaining groups' matmuls
                for m in range(MYS // 128):
                    for n in range(DIM // 512):
                        fp = fps[(m, n)]
                        for j in range(NCORES):
                            nc.tensor.matmul(
                                fp[:],
                                ag_sb[1][:, j, m * 128:(m + 1) * 128],
                                woT_sb[:, 2 * j + 1, n * 512:(n + 1) * 512],
                                start=False, stop=(j == NCORES - 1),
                                skip_group_check=True)
                        fo = fnp.tile([128, 512], BF16, tag="fo")
                        nc.vector.tensor_tensor(
                            fo[:], fp[:],
                            wobbc_sb[:, n * 512:(n + 1) * 512],
                            mybir.AluOpType.add)
                        nc.sync.dma_start(
                            out_e[m * 128:(m + 1) * 128,
                                  n * 512:(n + 1) * 512], fo[:])

    nc.compile()
    return nc


def _host_prep(x, rope_cache, wq_w, wq_b, wk_w, wk_b, wv_w, wv_b,
               wo_w, wo_b, sinks):
    """Build the per-core input maps (sharding + layout prep)."""
    xT = np.asarray(x, np.float32).reshape(S, DIM).T  # [DIM, S]
    xT = np.ascontiguousarray(xT).astype(ml_dtypes.bfloat16)
    # per seq block: [128, ND, SB] partition-major over the DIM axis
    xTs = []
    for s in range(NSB):
        blk = xT[:, s * SB:(s + 1) * SB]                    # [DIM, SB]
        blk = np.ascontiguousarray(
            blk.reshape(ND, 128, SB).transpose(1, 0, 2).reshape(128, ND * SB))
        xTs.append(blk)

    # rope tables in [hd, seq] layout; rows = hd index within the 128-row
    # chunk (2 heads of 64); sin sign folded (-, +) per 32-row half
    cos = np.asarray(rope_cache[:, :HD // 2], np.float32)   # [S, 32]
    sin = np.asarray(rope_cache[:, HD // 2:], np.float32)
    fidx = np.arange(128) % 32
    sign = np.where((np.arange(128) % 64) < 32, -1.0, 1.0)[:, None]
    cosT = cos.T[fidx]
    sinsT = sin.T[fidx] * sign

    # rope permutation: rot[m] = qk[perm(m)], perm swaps 32-halves per head
    perm = np.zeros((128, 128), np.float32)
    for m in range(128):
        p = m + 32 if (m % 64) < 32 else m - 32
        perm[p, m] = 1.0

    # diagonal staircase mask: keep q-col c >= k-row p within the tile
    maskd = np.triu(np.ones((128, 128), np.float32))
    ident = np.eye(128, dtype=np.float32)

    woT = np.asarray(wo_w, np.float32).T.astype(ml_dtypes.bfloat16)
    woT = np.ascontiguousarray(
        woT.reshape(ND, 128, DIM).transpose(1, 0, 2).reshape(128, ND * DIM))
    wob = np.asarray(wo_b, np.float32).astype(
        ml_dtypes.bfloat16).reshape(1, DIM)

    in_maps = []
    for c in range(NCORES):
        qsl = slice(c * HPC * HD, (c + 1) * HPC * HD)
        ksl = slice(c * HD, (c + 1) * HD)
        # weight-stationary chunks: chunk0 = q heads 0,1; chunk1 = q heads
        # 2,3; chunk2 = [k; v]. lhsT tile (c, d) = W_chunk[:, dim_d].T
        wq = np.asarray(wq_w, np.float32)[qsl]              # [256, DIM]
        wk = np.asarray(wk_w, np.float32)[ksl]              # [64, DIM]
        wv = np.asarray(wv_w, np.float32)[ksl]
        chunks = [wq[0:128], wq[128:256], np.concatenate([wk, wv], axis=0)]
        wqkvT = np.stack([ch.T.reshape(ND, 128, 128) for ch in chunks],
                         axis=0)                            # [3, ND, 128p, 128m]
        wqkvT = np.ascontiguousarray(
            wqkvT.transpose(2, 0, 1, 3).reshape(128, 3 * ND * 128)).astype(
                ml_dtypes.bfloat16)
        bias = np.concatenate([
            np.asarray(wq_b, np.float32)[qsl],
            np.asarray(wk_b, np.float32)[ksl],
            np.asarray(wv_b, np.float32)[ksl]])
        biasc = np.zeros((128, 16), np.float32)
        biasc[:, 0:3] = bias.reshape(3, 128).T
        wob_bc = np.tile(np.asarray(wo_b, np.float32).reshape(1, DIM),
                         (128, 1))
        consts = np.concatenate(
            [ident, perm, maskd, biasc, cosT, sinsT, wob_bc],
            axis=1).astype(ml_dtypes.bfloat16)
        assert consts.shape == (128, CO_END)
        sinks4 = np.ascontiguousarray(
            np.asarray(sinks, np.float32)[c * HPC:(c + 1) * HPC]).reshape(1, HPC)
        biasf = np.zeros((128, 4), np.float32)
        biasf[:, 0:3] = bias.reshape(3, 128).T
        im = {
            "sinks4": sinks4, "biasf": biasf,
            "consts": np.ascontiguousarray(consts),
            "wqkvT": wqkvT, "woT": woT,
        }
        for s in range(NSB):
            im[f"xT{s}"] = xTs[s]
        in_maps.append(im)
    return in_maps


def kernel(**inputs):
    global last_exec_time_ns
    if "nc" not in _cache:
        _cache["nc"] = _build()
    nc = _cache["nc"]
    in_maps = _host_prep(**inputs)
    trace = bool(int(os.environ.get("BASS_KERNEL_TRACE", "0")))
    if trace:
        try:
            _install_ntff_shim()
        except Exception:
            trace = False
    tc_env = os.environ.get("BASS_KERNEL_TRACE_CORES")
    kw = {}
    if trace and tc_env:
        kw["trace_cores"] = [int(c) for c in tc_env.split(",")]
    res = run_bass_kernel_spmd(nc, in_maps, core_ids=list(range(NCORES)),
                               trace=trace, **kw)
    last_exec_time_ns = res.exec_time_ns
    out = np.concatenate([np.asarray(res.results[c]["out"], np.float32)
                          for c in range(NCORES)], axis=0)
    return out.reshape(B, S, NH * HD)


# revision 60
# speedup vs baseline: 1.2792x; 1.2792x over previous
"""Trainium2 Bass kernel for GQA attention with RoPE, causal mask, and
attention sinks (nn_Attention_65094524338392).

Sharding: tensor-parallel by heads across 8 NeuronCores. Core c owns query
heads 4c..4c+3 and kv-head c (NREP=4). Each core computes QKV projections
over the full sequence for its heads directly in [hd, seq] layout
(weight-stationary matmuls against host-pretransposed x^T), flash-style
causal attention, then two chunked AllToAlls (one per head pair)
redistribute attention outputs from head-sharding to sequence-sharding so
each core computes the output projection for its 256-row sequence slice.
The first AllToAll overlaps with attention for the second head pair.

RoPE in [hd, seq] layout: rot(x) is a fixed 128-row permutation (swap
32-row halves within each head), computed as a permutation matmul on the
PE; then q = x*cosT + rot*sinsT with per-(row,col) cos/sin tables (sign
of sin folded into sinsT).

Math note: the sink scaling folds into the softmax normalizer:
    out = (sum_k exp(s_k) v_k) / (sum_k exp(s_k) + exp(sink))
so no logs/sigmoids are needed on device; |s| is small enough that no
max-subtraction is needed for exp stability.

Phases run sequentially and dense (no interleave): gaps in the PE stream
re-throttle the HAM clock gate (2.4 -> 1.2 GHz), so each phase boundary
re-pins the clock with a short burst of dummy matmuls.
"""

import os
import sys

sys.path.insert(0, "/opt/trn_rl_repo")

import ml_dtypes
import numpy as np

import concourse.bass as bass
import concourse.mybir as mybir
import concourse.tile as tile
from concourse import bacc
from concourse.bass_utils import run_bass_kernel_spmd

# Problem shapes
B, S, DIM = 1, 2048, 2048
NH, NKV, HD = 32, 8, 64
NREP = NH // NKV
SCALE = 1.0 / float(np.sqrt(HD))
NCORES = 8
HPC = NH // NCORES            # query heads per core (4)
SB = 512                      # seq block (attention q-block)
NSB = S // SB                 # 4
NT = S // 128                 # 16 seq tiles
ND = DIM // 128               # 16 contraction tiles
MYS = S // NCORES             # output rows per core (256)

# packed consts layout (bf16, 128 partitions): offsets in the free dim
CO_IDENT = 0
CO_PERM = 128
CO_MASKD = 256
CO_BIAS = 384          # [128, 16]; cols 0..2 = per-partition bias per chunk
CO_COS = 400
CO_SINS = 400 + S
CO_WOB = 400 + 2 * S   # [128, DIM]; wob broadcast across partitions
CO_END = 400 + 2 * S + DIM  # 6496

F32 = mybir.dt.float32
BF16 = mybir.dt.bfloat16

_cache = {}

last_exec_time_ns = None


def _install_ntff_shim():
    """Register the NTFF profile hook so trace=True yields exec_time_ns."""
    import types
    if "antenv.axon_hooks" in sys.modules:
        return
    import antenv
    mod = types.ModuleType("antenv.axon_hooks")
    mod._hook = None
    mod.set_axon_ntff_profile_hook = lambda h: setattr(mod, "_hook", h)
    mod.get_axon_ntff_profile_hook = lambda: mod._hook
    sys.modules["antenv.axon_hooks"] = mod
    antenv.axon_hooks = mod
    from trn_agent_boot.trn_boot import _ntff_profile_via_ctypes
    hook = _ntff_profile_via_ctypes("/opt/axon/libaxon_pjrt.so")
    if hook is not None:
        mod._hook = hook


def _build():
    nc = bacc.Bacc("TRN2", target_bir_lowering=False, debug=False,
                   num_devices=NCORES)

    # Input staging happens in declaration order; the packed consts and
    # wqkv come first, then x^T split by seq block so the first projection
    # can start early; woT (needed last) goes last.
    sinks_e = nc.declare_dram_parameter("sinks4", [1, HPC], F32, isOutput=False)
    biasf_e = nc.declare_dram_parameter("biasf", [128, 4], F32, isOutput=False)
    consts_e = nc.declare_dram_parameter("consts", [128, CO_END], BF16,
                                         isOutput=False)
    wqkvT_e = nc.declare_dram_parameter("wqkvT", [128, 3 * ND * 128], BF16,
                                        isOutput=False)
    xT_es = [nc.declare_dram_parameter(f"xT{s}", [128, ND * SB], BF16,
                                       isOutput=False) for s in range(NSB)]
    wob_e = nc.declare_dram_parameter("wob", [1, DIM], BF16, isOutput=False)
    woT_e = nc.declare_dram_parameter("woT", [128, ND * DIM], BF16, isOutput=False)
    out_e = nc.declare_dram_parameter("out", [MYS, DIM], BF16, isOutput=True)

    with tile.TileContext(nc) as tc:
        with tc.tile_pool(name="const", bufs=1) as cp, \
             tc.tile_pool(name="xT", bufs=2) as xtp, \
             tc.tile_pool(name="qk", bufs=3) as qkp, \
             tc.tile_pool(name="rope", bufs=2) as rp, \
             tc.tile_pool(name="pt", bufs=3) as ptp, \
             tc.tile_pool(name="ep", bufs=2) as epp, \
             tc.tile_pool(name="fin", bufs=3) as fnp, \
             tc.tile_pool(name="dram", bufs=1, space="DRAM") as dp:

            # ---- constants: one packed load + the tiny fp32 sinks ----
            sinks_sb = cp.tile([1, HPC], F32)
            nc.sync.dma_start(sinks_sb[:], sinks_e[:])
            biasf_sb = cp.tile([128, 4], F32)
            nc.sync.dma_start(biasf_sb[:], biasf_e[:])
            consts_sb = cp.tile([128, CO_END], BF16)
            nc.sync.dma_start(consts_sb[:], consts_e[:])
            ident_sb = consts_sb[:, CO_IDENT:CO_IDENT + 128]
            perm_sb = consts_sb[:, CO_PERM:CO_PERM + 128]
            maskd_sb = consts_sb[:, CO_MASKD:CO_MASKD + 128]
            cosT_sb = consts_sb[:, CO_COS:CO_COS + S]
            sinsT_sb = consts_sb[:, CO_SINS:CO_SINS + S]
            wobbc_sb = consts_sb[:, CO_WOB:CO_WOB + DIM]
            wqkv_sb = cp.tile([128, 3, ND, 128], BF16)
            nc.sync.dma_start(wqkv_sb[:], wqkvT_e[:].rearrange(
                "p (c d f) -> p c d f", c=3, d=ND))
            # x^T per seq block; first 2 issued up front, the rest inside
            # the projection loop (2 bufs, one block of prefetch)
            xT_s = []
            for s in range(NSB):
                t = xtp.tile([128, ND, SB], BF16, tag="xT")
                if s < 2:
                    nc.sync.dma_start(t[:], xT_es[s][:].rearrange(
                        "p (d f) -> p d f", d=ND))
                xT_s.append(t)
            wob_sb = cp.tile([1, DIM], BF16)
            nc.sync.dma_start(wob_sb[:], wob_e[:])

            es_sb = cp.tile([1, HPC], F32)
            nc.scalar.activation(es_sb[:], sinks_sb[:],
                                 mybir.ActivationFunctionType.Exp)
            ones512 = cp.tile([1, 512], BF16)
            nc.gpsimd.memset(ones512[:], 1.0)

            # PE warm-up: a gapless burst of dummy matmuls during the
            # initial input DMAs releases the HAM clock throttle (1.2 ->
            # 2.4 GHz) before real work begins. No data dependencies.
            warm_sb = cp.tile([128, 512], BF16)
            nc.gpsimd.memset(warm_sb[:], 0.0)
            with tc.tile_pool(name="warm", bufs=1, space="PSUM") as wpp:
                warm_ps = wpp.tile([128, 512], F32, tag="warm")
                for _ in range(16):
                    nc.tensor.matmul(warm_ps[:], warm_sb[:, 0:128],
                                     warm_sb[:], start=True, stop=True)

            # persistent activations
            qP = [cp.tile([128, S], BF16, name=f"qP{g}") for g in range(HPC // 2)]
            kTd = cp.tile([128, S], BF16)
            # v padded to 128 columns (ones col + zeros) so the PV matmul's
            # stationary load is FWL-eligible (NumWeights==128)
            v_sb = cp.tile([128, NT, 128], BF16)
            nc.gpsimd.memset(v_sb[:, :, HD:HD + 1], 1.0)
            nc.gpsimd.memset(v_sb[:, :, HD + 1:], 0.0)
            oT = [cp.tile([128, S], BF16, name=f"oT{g}") for g in range(HPC // 2)]
            woT_sb = cp.tile([128, ND, DIM], BF16)

            a2a_in = [dp.tile([S // 2, MYS], BF16, name=f"a2ai{g}")
                      for g in range(2)]
            a2a_out = [dp.tile([S // 2, MYS], BF16, name=f"a2ao{g}")
                      for g in range(2)]

            # ---- phase B: QKV projections + rope -> qP/kTd/v_sb ----
            with tc.tile_pool(name="ppB", bufs=1, space="PSUM") as ppB:
                # dependency-gated warm-up: fires as soon as the first
                # x^T tile lands so QKV starts at the warm PE clock
                warm2 = ppB.tile([128, 512], F32, tag="rot", bufs=2)
                for _ in range(12):
                    nc.tensor.matmul(warm2[:], xT_s[0][:, 0, 0:128],
                                     warm_sb[:], start=True, stop=True)
                for s in range(NSB):
                    if s < 2:
                        nc.sync.dma_start(
                            xT_s[s + 2][:], xT_es[s + 2][:].rearrange(
                                "p (d f) -> p d f", d=ND))
                    sl = slice(s * SB, (s + 1) * SB)
                    for c in range(3):
                        acc = ppB.tile([128, SB], F32, tag="acc", bufs=3)
                        for d in range(ND):
                            nc.tensor.matmul(acc[:], wqkv_sb[:, c, d, :],
                                             xT_s[s][:, d, :],
                                             start=(d == 0),
                                             stop=(d == ND - 1))
                        # bias add (per-partition) fused into the PSUM->SBUF
                        # move on the DVE
                        qk = qkp.tile([128, SB], BF16, tag="qk")
                        nc.vector.tensor_scalar_add(qk[:], acc[:],
                                                    biasf_sb[:, c:c + 1])
                        # rope: rot = perm @ qk (PE), q = qk*cos + rot*sins
                        rot = ppB.tile([128, 512], F32, tag="rot", bufs=2)
                        nc.tensor.matmul(rot[:], perm_sb, qk[:],
                                         start=True, stop=True)
                        t1 = rp.tile([128, SB], BF16, tag="t1")
                        nc.vector.tensor_tensor(t1[:], qk[:], cosT_sb[:, sl],
                                                mybir.AluOpType.mult)
                        t2 = rp.tile([128, SB], BF16, tag="t2")
                        nc.vector.tensor_tensor(t2[:], rot[:], sinsT_sb[:, sl],
                                                mybir.AluOpType.mult)
                        if c < 2:
                            nc.vector.tensor_tensor(qP[c][:, sl], t1[:], t2[:],
                                                    mybir.AluOpType.add)
                        else:
                            nc.vector.tensor_tensor(
                                kTd[0:HD, sl], t1[0:HD, :], t2[0:HD, :],
                                mybir.AluOpType.add)
                            nc.vector.tensor_copy(kTd[HD:128, sl],
                                                  kTd[0:HD, sl])
                            # v: transpose rows 64:128 of qk into [seq, hd]
                            vtg = ppB.tile([128, 256], BF16, tag="vt", bufs=2)
                            for tt in range(4):
                                nc.tensor.transpose(
                                    vtg[:, tt * 64:(tt + 1) * 64],
                                    qk[HD:128, tt * 128:(tt + 1) * 128],
                                    ident_sb[HD:128, HD:128])
                            nc.vector.tensor_copy(
                                v_sb[:, 4 * s:4 * s + 4, 0:HD],
                                vtg[:].rearrange("p (t f) -> p t f", t=4))

            def emit_pv(p0, n_kt, s, pvs, pts):
                """PV matmuls for the pair of p-iterations (p0, p0+1)."""
                for z in range(2):
                    for pp in (p0, p0 + 1):
                        for half in range(2):
                            i = 2 * pp + half
                            st = 128 * (i - 4 * s) if i >= 4 * s else 0
                            nc.tensor.matmul(
                                pvs[z][:, st:512],
                                v_sb[:, i, :],
                                pts[pp][:, z, half * 512 + st:
                                        (half + 1) * 512],
                                start=(i == 0),
                                stop=(i == n_kt - 1))

            def attn_block(ppC, g, s, warm_n):
                """Attention for head pair g over q block s."""
                n_kt = 4 * (s + 1)
                sl = slice(s * SB, (s + 1) * SB)
                if warm_n:
                    # re-pin the warm PE clock across the phase boundary
                    warm_w = ppC.tile([128, 1024], F32, tag="sc", bufs=3)
                    for _ in range(warm_n):
                        nc.tensor.matmul(warm_w[:, 0:512], warm_sb[:, 0:128],
                                         warm_sb[:], start=True, stop=True)
                pvs = [ppC.tile([128, 512], F32, tag="pv", bufs=2,
                                name=f"pv{g}_{s}_{z}") for z in range(2)]
                pts = []
                for p in range(n_kt // 2):
                    scs = []
                    for z in range(2):
                        sc = ppC.tile([128, 1024], F32, tag="sc", bufs=3,
                                      name=f"sc{g}_{s}_{p}_{z}")
                        for half in range(2):
                            i = 2 * p + half
                            st = 128 * (i - 4 * s) if i >= 4 * s else 0
                            nc.tensor.matmul(
                                sc[:, half * 512 + st:(half + 1) * 512],
                                kTd[z * HD:(z + 1) * HD,
                                    i * 128:(i + 1) * 128],
                                qP[g][z * HD:(z + 1) * HD,
                                      s * SB + st:(s + 1) * SB],
                                start=True, stop=True,
                                tile_position=(z * HD, 0))
                        scs.append(sc)
                    pt = ptp.tile([128, 2, 1024], BF16, tag="pt")
                    for z in range(2):
                        if p == 2 * s + 1:
                            # both halves heavily column-restricted: exp
                            # only the computed regions (saves ACT time)
                            for half in range(2):
                                i = 2 * p + half
                                st = 128 * (i - 4 * s)
                                nc.scalar.activation(
                                    pt[:, z, half * 512 + st:(half + 1) * 512],
                                    scs[z][:, half * 512 + st:(half + 1) * 512],
                                    mybir.ActivationFunctionType.Exp,
                                    scale=SCALE)
                        else:
                            nc.scalar.activation(
                                pt[:, z, :], scs[z][:],
                                mybir.ActivationFunctionType.Exp, scale=SCALE)
                    for z in range(2):
                        for half in range(2):
                            i = 2 * p + half
                            if i >= 4 * s:
                                st = 128 * (i - 4 * s)
                                nc.vector.tensor_tensor(
                                    pt[:, z, half * 512 + st:
                                       half * 512 + st + 128],
                                    pt[:, z, half * 512 + st:
                                       half * 512 + st + 128],
                                    maskd_sb,
                                    mybir.AluOpType.mult)
                    pts.append(pt)
                    if p % 2 == 1:
                        emit_pv(p - 1, n_kt, s, pvs, pts)

                for z in range(2):
                    h = 2 * g + z
                    pv = pvs[z]
                    # epilogue: out_h = pv[0:64] / (S_row + exp(sink_h))
                    srow = epp.tile([1, 512], F32, tag="srow", bufs=2)
                    nc.vector.scalar_tensor_tensor(
                        srow[:], pv[HD:HD + 1, :], es_sb[0:1, h:h + 1],
                        ones512[:], mybir.AluOpType.add,
                        mybir.AluOpType.mult)
                    rrow = epp.tile([1, 512], F32, tag="rrow", bufs=2)
                    nc.vector.reciprocal_approx_fast(rrow[:], srow[:])
                    rbc = epp.tile([HD, 512], F32, tag="rbc", bufs=3)
                    nc.gpsimd.partition_broadcast(rbc[:], rrow[0:1, :])
                    nc.vector.tensor_tensor(
                        oT[g][z * HD:(z + 1) * HD, sl], pv[0:HD, :],
                        rbc[:], mybir.AluOpType.mult)
                    # stream this (head, block) slice into its A2A shard
                    nc.sync.dma_start(
                        a2a_in[g][:].rearrange(
                            "(j zz p) n -> zz p j n",
                            j=NCORES, zz=2)[z][:, 2 * s:2 * s + 2],
                        oT[g][z * HD:(z + 1) * HD, sl].rearrange(
                            "p (j n) -> p j n", j=2))

            # ---- phase C: attention, head pair 0 then head pair 1 ----
            ag_sb = [cp.tile([128, NCORES, MYS], BF16, name=f"ag{g}")
                     for g in range(2)]
            with tc.tile_pool(name="ppC", bufs=1, space="PSUM") as ppC:
                for s in range(NSB):
                    attn_block(ppC, 0, s, 20 if s == 0 else 0)

                # first AllToAll: heads 0,1 -> sequence shards; overlaps
                # with attention for head pair 1
                nc.gpsimd.collective_compute(
                    "AllToAll", mybir.AluOpType.bypass,
                    replica_groups=[list(range(NCORES))],
                    ins=[a2a_in[0].opt()], outs=[a2a_out[0].opt()])
                nc.sync.dma_start(ag_sb[0][:], a2a_out[0][:].rearrange(
                    "(o p) n -> p o n", p=128))

                # load wo^T now on the SWDGE queue (idle; needed only by
                # the output projection)
                nc.gpsimd.dma_start(woT_sb[:], woT_e[:].rearrange(
                    "p (o f) -> p o f", o=ND))

                for s in range(NSB):
                    attn_block(ppC, 1, s, 6 if s == 0 else 0)

            nc.gpsimd.collective_compute(
                "AllToAll", mybir.AluOpType.bypass,
                replica_groups=[list(range(NCORES))],
                ins=[a2a_in[1].opt()], outs=[a2a_out[1].opt()])
            ago = a2a_out[1][:].rearrange("(o p) n -> p o n", p=128)
            for q in range(4):
                nc.sync.dma_start(ag_sb[1][:, 2 * q:2 * q + 2, :],
                                  ago[:, 2 * q:2 * q + 2, :])

            # ---- output projection for my sequence slice ----
            # contraction rows: kt = 2*j + g -> a2a_out[g] shard j.
            # Two passes over kt parity: the even pass (head pair 0, from
            # the first AllToAll) runs during the second AllToAll's entry
            # barrier; the odd pass runs once its readback lands. All 8
            # accumulation groups hold their PSUM banks across the passes.
            with tc.tile_pool(name="ppD", bufs=1, space="PSUM") as ppD:
                fps = {}
                for m in range(MYS // 128):
                    for n in range(DIM // 512):
                        fps[(m, n)] = ppD.tile([128, 512], F32, tag="fp",
                                               bufs=8, name=f"fp{m}_{n}")
                # re-warm the PE into the first two banks (results are
                # discarded by the start=True of each even pass)
                for mn in ((0, 0), (0, 1)):
                    for _ in range(7):
                        nc.tensor.matmul(fps[mn][:], warm_sb[:, 0:128],
                                         warm_sb[:], start=True, stop=True)
                # even pass: head-pair-0 contraction tiles (first AllToAll)
                for m in range(MYS // 128):
                    for n in range(DIM // 512):
                        fp = fps[(m, n)]
                        for j in range(NCORES):
                            nc.tensor.matmul(
                                fp[:],
                                ag_sb[0][:, j, m * 128:(m + 1) * 128],
                                woT_sb[:, 2 * j, n * 512:(n + 1) * 512],
                                start=(j == 0), stop=False)
                # keep the PE clock warm across the second AllToAll wait:
                # accumulate +0 (zero stationary) into an open group; the
                # second batch gates on the first readback chunk so it runs
                # right before the odd pass
                for _ in range(20):
                    nc.tensor.matmul(fps[(0, 0)][:], warm_sb[:, 0:128],
                                     warm_sb[:], start=False, stop=False,
                                     skip_group_check=True)
                for _ in range(16):
                    nc.tensor.matmul(
                        fps[(0, 1)][:], warm_sb[:, 0:128],
                        ag_sb[1][:, 0:2, :].rearrange("p a b -> p (a b)"),
                        start=False, stop=False, skip_group_check=True)
                # odd pass: j outer so each readback chunk unblocks early
                for j in range(NCORES):
                    for m in range(MYS // 128):
                        for n in range(DIM // 512):
                            nc.tensor.matmul(
                                fps[(m, n)][:],
                                ag_sb[1][:, j, m * 128:(m + 1) * 128],
                                woT_sb[:, 2 * j + 1, n * 512:(n + 1) * 512],
                                start=False, stop=(j == NCORES - 1),
                                skip_group_check=True)
                for m in range(MYS // 128):
                    for n in range(DIM // 512):
                        fp = fps[(m, n)]
                        fo = fnp.tile([128, 512], BF16, tag="fo")
                        nc.vector.tensor_tensor(
                            fo[:], fp[:],
                            wobbc_sb[:, n * 512:(n + 1) * 512],
                            mybir.AluOpType.add)
                        nc.sync.dma_start(
                            out_e[m * 128:(m + 1) * 128,
                                  n * 512:(n + 1) * 512], fo[:])

    nc.compile()
    return nc


def _host_prep(x, rope_cache, wq_w, wq_b, wk_w, wk_b, wv_w, wv_b,
               wo_w, wo_b, sinks):
    """Build the per-core input maps (sharding + layout prep)."""
    xT = np.asarray(x, np.float32).reshape(S, DIM).T  # [DIM, S]
    xT = np.ascontiguousarray(xT).astype(ml_dtypes.bfloat16)
    # per seq block: [128, ND, SB] partition-major over the DIM axis
    xTs = []
    for s in range(NSB):
        blk = xT[:, s * SB:(s + 1) * SB]                    # [DIM, SB]
        blk = np.ascontiguousarray(
            blk.reshape(ND, 128, SB).transpose(1, 0, 2).reshape(128, ND * SB))
        xTs.append(blk)

    # rope tables in [hd, seq] layout; rows = hd index within the 128-row
    # chunk (2 heads of 64); sin sign folded (-, +) per 32-row half
    cos = np.asarray(rope_cache[:, :HD // 2], np.float32)   # [S, 32]
    sin = np.asarray(rope_cache[:, HD // 2:], np.float32)
    fidx = np.arange(128) % 32
    sign = np.where((np.arange(128) % 64) < 32, -1.0, 1.0)[:, None]
    cosT = cos.T[fidx]
    sinsT = sin.T[fidx] * sign

    # rope permutation: rot[m] = qk[perm(m)], perm swaps 32-halves per head
    perm = np.zeros((128, 128), np.float32)
    for m in range(128):
        p = m + 32 if (m % 64) < 32 else m - 32
        perm[p, m] = 1.0

    # diagonal staircase mask: keep q-col c >= k-row p within the tile
    maskd = np.triu(np.ones((128, 128), np.float32))
    ident = np.eye(128, dtype=np.float32)

    woT = np.asarray(wo_w, np.float32).T.astype(ml_dtypes.bfloat16)
    woT = np.ascontiguousarray(
        woT.reshape(ND, 128, DIM).transpose(1, 0, 2).reshape(128, ND * DIM))
    wob = np.asarray(wo_b, np.float32).astype(
        ml_dtypes.bfloat16).reshape(1, DIM)

    in_maps = []
    for c in range(NCORES):
        qsl = slice(c * HPC * HD, (c + 1) * HPC * HD)
        ksl = slice(c * HD, (c + 1) * HD)
        # weight-stationary chunks: chunk0 = q heads 0,1; chunk1 = q heads
        # 2,3; chunk2 = [k; v]. lhsT tile (c, d) = W_chunk[:, dim_d].T
        wq = np.asarray(wq_w, np.float32)[qsl]              # [256, DIM]
        wk = np.asarray(wk_w, np.float32)[ksl]              # [64, DIM]
        wv = np.asarray(wv_w, np.float32)[ksl]
        chunks = [wq[0:128], wq[128:256], np.concatenate([wk, wv], axis=0)]
        wqkvT = np.stack([ch.T.reshape(ND, 128, 128) for ch in chunks],
                         axis=0)                            # [3, ND, 128p, 128m]
        wqkvT = np.ascontiguousarray(
            wqkvT.transpose(2, 0, 1, 3).reshape(128, 3 * ND * 128)).astype(
                ml_dtypes.bfloat16)
        bias = np.concatenate([
            np.asarray(wq_b, np.float32)[qsl],
            np.asarray(wk_b, np.float32)[ksl],
            np.asarray(wv_b, np.float32)[ksl]])
        biasc = np.zeros((128, 16), np.float32)
        biasc[:, 0:3] = bias.reshape(3, 128).T
        wob_bc = np.tile(np.asarray(wo_b, np.float32).reshape(1, DIM),
                         (128, 1))
        consts = np.concatenate(
            [ident, perm, maskd, biasc, cosT, sinsT, wob_bc],
            axis=1).astype(ml_dtypes.bfloat16)
        assert consts.shape == (128, CO_END)
        sinks4 = np.ascontiguousarray(
            np.asarray(sinks, np.float32)[c * HPC:(c + 1) * HPC]).reshape(1, HPC)
        biasf = np.zeros((128, 4), np.float32)
        biasf[:, 0:3] = bias.reshape(3, 128).T
        im = {
            "sinks4": sinks4, "biasf": biasf,
            "consts": np.ascontiguousarray(consts),
            "wqkvT": wqkvT, "wob": wob, "woT": woT,
        }
        for s in range(NSB):
            im[f"xT{s}"] = xTs[s]
        in_maps.append(im)
    return in_maps


def kernel(**inputs):
    global last_exec_time_ns
    if "nc" not in _cache:
        _cache["nc"] = _build()
    nc = _cache["nc"]
    in_maps = _host_prep(**inputs)
    trace = bool(int(os.environ.get("BASS_KERNEL_TRACE", "0")))
    if trace:
        try:
            _install_ntff_shim()
        except Exception:
            trace = False
    tc_env = os.environ.get("BASS_KERNEL_TRACE_CORES")
    kw = {}
    if trace and tc_env:
        kw["trace_cores"] = [int(c) for c in tc_env.split(",")]
    res = run_bass_kernel_spmd(nc, in_maps, core_ids=list(range(NCORES)),
                               trace=trace, **kw)
    last_exec_time_ns = res.exec_time_ns
    out = np.concatenate([np.asarray(res.results[c]["out"], np.float32)
                          for c in range(NCORES)], axis=0)
    return out.reshape(B, S, NH * HD)
